# revision 4
# baseline (speedup 1.0000x reference)
"""FFN-in-head attention Trainium2 kernel (8 NeuronCores, SPMD).

Sharding: core = (batch b, token-half). Each core runs the q/k/v MLPs for its
1024 tokens (channel-major outputs via fp32r matmuls), pairwise-AllGathers
K (channel-major fp32r) and V (token-major bf16) between the two cores of a
batch, computes all 16 heads of attention for its 1024 query tokens
(logits transposed [keys, queries] so the exp'd probabilities are directly in
matmul-stationary layout; softmax denominator via a ones-column appended to V),
and finishes with the output projection (bias folded in as a K=1 matmul).
"""
import sys
sys.path.insert(0, "/opt/trn_rl_repo")
import numpy as np

DIM = 1024
HEADS = 16
HD = 64
HIDDEN = 4096
B = 4
N = 2048
NT = 1024          # tokens per core
P = 128
N_CORES = 8
CH = 2             # token chunks per core in the MLP phase
CHT = NT // CH     # 512 tokens per chunk
HT = HIDDEN // P   # 32 hidden tiles
CT = DIM // P      # 8 channel tiles
KT = N // P        # 16 key tiles
QT = NT // P       # 8 query-token tiles


def build_program():
    import concourse.bass as bass
    import concourse.mybir as mybir
    import concourse.tile as tile
    from concourse import bacc
    from concourse.masks import make_identity

    fp32 = mybir.dt.float32
    fp32r = mybir.dt.float32r
    bf16 = mybir.dt.bfloat16
    AF = mybir.ActivationFunctionType

    nc = bacc.Bacc("TRN2", target_bir_lowering=False, debug=False,
                   num_devices=N_CORES)

    # ---- DRAM I/O ----
    xT = nc.dram_tensor("xT", [DIM, NT], fp32r, kind="ExternalInput").ap()
    w1s, b1s, w2s = {}, {}, {}
    for m in ("q", "k", "v"):
        w1s[m] = nc.dram_tensor(f"{m}_w1", [DIM, HIDDEN], fp32r, kind="ExternalInput").ap()
        b1s[m] = nc.dram_tensor(f"{m}_b1r", [P, HT], fp32, kind="ExternalInput").ap()
        w2s[m] = nc.dram_tensor(f"{m}_w2", [HIDDEN, DIM], fp32r, kind="ExternalInput").ap()
    qb2 = nc.dram_tensor("q_b2r", [P, CT], fp32, kind="ExternalInput").ap()
    kb2 = nc.dram_tensor("k_b2r", [P, CT], fp32, kind="ExternalInput").ap()
    vb2 = nc.dram_tensor("v_b2r", [1, DIM], fp32r, kind="ExternalInput").ap()
    pw = nc.dram_tensor("proj_w", [DIM, DIM], fp32r, kind="ExternalInput").ap()
    pbr = nc.dram_tensor("proj_br", [1, DIM], fp32r, kind="ExternalInput").ap()
    ones_r = nc.dram_tensor("ones_r", [1, P], fp32r, kind="ExternalInput").ap()
    out = nc.dram_tensor("out", [NT, DIM], fp32, kind="ExternalOutput").ap()

    with tile.TileContext(nc) as tc:
        with tc.tile_pool(name="outer", bufs=1) as outer, \
             tc.tile_pool(name="ps", bufs=4, space="PSUM") as psp, \
             tc.tile_pool(name="ps2", bufs=2, space="PSUM") as psp2, \
             tc.tile_pool(name="dram", bufs=1, space="DRAM") as dram:

            # long-lived SBUF
            xT_sb = outer.tile([P, CT, NT], fp32r, tag="big")
            nc.sync.dma_start(xT_sb[:], xT.rearrange("(ct p) t -> p ct t", p=P))
            qT_sb = outer.tile([P, CT, NT], fp32r, tag="qT")
            ones_sb = outer.tile([1, P], fp32r, tag="ones")
            nc.sync.dma_start(ones_sb[:], ones_r[:])
            vb2_sb = outer.tile([1, DIM], fp32r, tag="vb2")
            nc.sync.dma_start(vb2_sb[:], vb2[:])
            pbr_sb = outer.tile([1, DIM], fp32r, tag="pbr")
            nc.sync.dma_start(pbr_sb[:], pbr[:])

            # DRAM bounce buffers for collectives
            ag_k_in = dram.tile([DIM, NT], fp32r, tag="agki")
            ag_k_out = dram.tile([2 * DIM, NT], fp32r, tag="agko")
            ag_v_in = dram.tile([NT, DIM], bf16, tag="agvi")
            ag_v_out = dram.tile([2 * NT, DIM], bf16, tag="agvo")

            # ======== MLP phase ========
            with tc.tile_pool(name="mlp", bufs=1) as mlp, \
                 tc.tile_pool(name="w1p", bufs=2) as w1p, \
                 tc.tile_pool(name="w2p", bufs=2) as w2p, \
                 tc.tile_pool(name="w2vp", bufs=3) as w2vp, \
                 tc.tile_pool(name="stg", bufs=3) as stg:

                for m in ("k", "v", "q"):
                    b1_sb = mlp.tile([P, HT], fp32, tag="b1")
                    nc.sync.dma_start(b1_sb[:], b1s[m][:])
                    if m in ("k", "q"):
                        b2_sb = mlp.tile([P, CT], fp32, tag="b2")
                        nc.sync.dma_start(b2_sb[:], (kb2 if m == "k" else qb2)[:])
                    w1_r = w1s[m].rearrange("(kc p) h -> p kc h", p=P)
                    w2_r = w2s[m].rearrange("(kh p) c -> p kh c", p=P)

                    for ch in range(CH):
                        tok = slice(ch * CHT, (ch + 1) * CHT)
                        # ---- fc1: hT[h, tok_chunk] = gelu(w1.T @ x + b1)
                        hT_sb = mlp.tile([P, HT, CHT], fp32r, tag="hT")
                        for ht in range(HT):
                            w1t = w1p.tile([P, CT, P], fp32r, tag="w1t")
                            nc.sync.dma_start(
                                w1t[:], w1_r[:, :, ht * P:(ht + 1) * P])
                            ps = psp.tile([P, CHT], mybir.dt.float32, tag="ps")
                            for kc in range(CT):
                                nc.tensor.matmul(
                                    ps[:], w1t[:, kc, :], xT_sb[:, kc, tok],
                                    start=(kc == 0), stop=(kc == CT - 1))
                            nc.scalar.activation(
                                hT_sb[:, ht, :], ps[:], AF.Gelu,
                                bias=b1_sb[:, ht:ht + 1])

                        if m in ("k", "q"):
                            # ---- fc2 channel-major: y[c, tok] = w2.T @ h + b2
                            for ct in range(CT):
                                ps = psp.tile([P, CHT], mybir.dt.float32, tag="ps")
                                for khh in range(2):
                                    w2t = w2p.tile([P, HT // 2, P], fp32r, tag="w2t")
                                    nc.sync.dma_start(
                                        w2t[:],
                                        w2_r[:, khh * (HT // 2):(khh + 1) * (HT // 2),
                                             ct * P:(ct + 1) * P])
                                    for kh in range(HT // 2):
                                        k_abs = khh * (HT // 2) + kh
                                        nc.tensor.matmul(
                                            ps[:], w2t[:, kh, :], hT_sb[:, k_abs, :],
                                            start=(k_abs == 0), stop=(k_abs == HT - 1))
                                if m == "q":
                                    nc.scalar.activation(
                                        qT_sb[:, ct, tok], ps[:], AF.Identity,
                                        bias=b2_sb[:, ct:ct + 1])
                                else:
                                    kst = stg.tile([P, CHT], fp32r, tag="kst")
                                    nc.scalar.activation(
                                        kst[:], ps[:], AF.Identity,
                                        bias=b2_sb[:, ct:ct + 1])
                                    nc.sync.dma_start(
                                        ag_k_in[ct * P:(ct + 1) * P, tok], kst[:])
                        else:
                            # ---- fc2 token-major (v): y[tok, c] = h.T @ w2 + b2
                            TT = CHT // P  # 4 token tiles per chunk
                            for nt in range(2):
                                cs = slice(nt * 512, (nt + 1) * 512)
                                pss = [psp.tile([P, 512], mybir.dt.float32, tag="ps",
                                                name=f"psv{tt}")
                                       for tt in range(TT)]
                                for kh in range(HT):
                                    w2vt = w2vp.tile([P, 512], fp32r, tag="w2vt")
                                    nc.sync.dma_start(w2vt[:], w2_r[:, kh, cs])
                                    for tt in range(TT):
                                        nc.tensor.matmul(
                                            pss[tt][:],
                                            hT_sb[:, kh, tt * P:(tt + 1) * P],
                                            w2vt[:],
                                            start=(kh == 0), stop=False,
                                            skip_group_check=True)
                                for tt in range(TT):
                                    nc.tensor.matmul(
                                        pss[tt][:], ones_sb[:, :], vb2_sb[:, cs],
                                        start=False, stop=True,
                                        skip_group_check=True)
                                    vst = stg.tile([P, 512], bf16, tag="vst")
                                    nc.vector.tensor_copy(vst[:], pss[tt][:])
                                    nc.sync.dma_start(
                                        ag_v_in[ch * CHT + tt * P:
                                                ch * CHT + (tt + 1) * P, cs],
                                        vst[:])

                    # kick off the pairwise AllGather as soon as k / v done
                    if m == "k":
                        nc.gpsimd.collective_compute(
                            "AllGather", mybir.AluOpType.bypass,
                            replica_groups=[[0, 1], [2, 3], [4, 5], [6, 7]],
                            ins=[ag_k_in.opt()], outs=[ag_k_out.opt()])
                    elif m == "v":
                        nc.gpsimd.collective_compute(
                            "AllGather", mybir.AluOpType.bypass,
                            replica_groups=[[0, 1], [2, 3], [4, 5], [6, 7]],
                            ins=[ag_v_in.opt()], outs=[ag_v_out.opt()])

            # ======== attention phase ========
            aoT_sb = outer.tile([P, CT, NT], fp32r, tag="big")
            k_r = ag_k_out.rearrange("(half ct p) t -> p ct half t",
                                     half=2, ct=CT, p=P)
            v_r = ag_v_out.rearrange("(ktl p) c -> p ktl c", p=P)
            with tc.tile_pool(name="attn", bufs=2) as atp, \
                 tc.tile_pool(name="attbig", bufs=1) as atbig, \
                 tc.tile_pool(name="attsm", bufs=4) as smp, \
                 tc.tile_pool(name="ident", bufs=1) as idp:
                ident = idp.tile([P, P], fp32, tag="id")
                make_identity(nc, ident[:])
                for hp in range(HEADS // 2):
                    k2 = atp.tile([P, 2, NT], fp32r, tag="k2")
                    nc.sync.dma_start(k2[:], k_r[:, hp])
                    vA = atp.tile([P, KT, HD + 1], bf16, tag="vA")
                    vB = atp.tile([P, KT, HD + 1], bf16, tag="vB")
                    cA, cB = 2 * hp * HD, (2 * hp + 1) * HD
                    nc.sync.dma_start(vA[:, :, 0:HD], v_r[:, :, cA:cA + HD])
                    nc.sync.dma_start(vB[:, :, 0:HD], v_r[:, :, cB:cB + HD])
                    nc.gpsimd.memset(vA[:, :, HD:HD + 1], 1.0)
                    nc.gpsimd.memset(vB[:, :, HD:HD + 1], 1.0)

                    atA = atbig.tile([P, KT, NT], bf16, tag="atA")
                    atB = atbig.tile([P, KT, NT], bf16, tag="atB")
                    for ktl in range(KT):
                        half, col = ktl // CT, (ktl % CT) * P
                        psA = psp2.tile([P, NT], mybir.dt.float32, tag="ps2")
                        psB = psp2.tile([P, NT], mybir.dt.float32, tag="ps2")
                        for nt in range(2):
                            qs = slice(nt * 512, (nt + 1) * 512)
                            nc.tensor.matmul(
                                psA[:, qs], k2[0:HD, half, col:col + P],
                                qT_sb[0:HD, hp, qs], start=True, stop=True,
                                skip_group_check=True)
                            nc.tensor.matmul(
                                psB[:, qs], k2[HD:P, half, col:col + P],
                                qT_sb[HD:P, hp, qs], start=True, stop=True,
                                skip_group_check=True)
                        nc.scalar.activation(atA[:, ktl, :], psA[:], AF.Exp,
                                             scale=float(HD) ** -0.5)
                        nc.scalar.activation(atB[:, ktl, :], psB[:], AF.Exp,
                                             scale=float(HD) ** -0.5)

                    ao = atp.tile([P, QT, P], fp32, tag="ao")
                    for head, at, vv in ((0, atA, vA), (1, atB, vB)):
                        for qtl in range(QT):
                            psv = psp.tile([P, HD + 1], mybir.dt.float32, tag="ps")
                            for ktl in range(KT):
                                nc.tensor.matmul(
                                    psv[:], at[:, ktl, qtl * P:(qtl + 1) * P],
                                    vv[:, ktl, :],
                                    start=(ktl == 0), stop=(ktl == KT - 1))
                            rec = smp.tile([P, 1], fp32, tag="rec")
                            nc.vector.reciprocal(rec[:], psv[:, HD:HD + 1])
                            nc.vector.tensor_scalar_mul(
                                ao[:, qtl, head * HD:(head + 1) * HD],
                                psv[:, 0:HD], rec[:])
                    # transpose ao block -> aoT channel-major
                    for qtl in range(QT):
                        pst = psp.tile([P, P], mybir.dt.float32, tag="ps")
                        nc.tensor.transpose(pst[:], ao[:, qtl, :], ident[:])
                        nc.vector.tensor_copy(
                            aoT_sb[:, hp, qtl * P:(qtl + 1) * P], pst[:])

            # ======== output projection ========
            with tc.tile_pool(name="proj", bufs=16) as pjp:
                pwt = {}
                for nt in range(2):
                    for kc in range(CT):
                        t = pjp.tile([P, 512], fp32r, tag="pwt")
                        nc.sync.dma_start(
                            t[:], pw[kc * P:(kc + 1) * P,
                                     nt * 512:(nt + 1) * 512])
                        pwt[(nt, kc)] = t
                for tt in range(QT):
                    for nt in range(2):
                        cs = slice(nt * 512, (nt + 1) * 512)
                        ps = psp.tile([P, 512], mybir.dt.float32, tag="ps")
                        for kc in range(CT):
                            nc.tensor.matmul(
                                ps[:], aoT_sb[:, kc, tt * P:(tt + 1) * P],
                                pwt[(nt, kc)][:],
                                start=(kc == 0), stop=False,
                                skip_group_check=True)
                        nc.tensor.matmul(
                            ps[:], ones_sb[:, :], pbr_sb[:, cs],
                            start=False, stop=True, skip_group_check=True)
                        ot = pjp.tile([P, 512], mybir.dt.float32, tag="ot",
                                      bufs=3)
                        nc.vector.tensor_copy(ot[:], ps[:])
                        nc.sync.dma_start(
                            out[tt * P:(tt + 1) * P, cs], ot[:])

    nc.compile()
    return nc


_CACHE = {}


def _get_runner():
    if "runner" in _CACHE:
        return _CACHE["runner"]
    import jax
    import numpy as np
    from jax.sharding import Mesh, PartitionSpec
    from jax.experimental.shard_map import shard_map
    from concourse import mybir
    from concourse.bass2jax import (_bass_exec_p, partition_id_tensor,
                                    install_neuronx_cc_hook)

    nc = build_program()
    install_neuronx_cc_hook()
    partition_name = nc.partition_id_tensor.name if nc.partition_id_tensor else None
    in_names, out_names, out_avals = [], [], []
    for alloc in nc.m.functions[0].allocations:
        if not isinstance(alloc, mybir.MemoryLocationSet):
            continue
        name = alloc.memorylocations[0].name
        if alloc.kind == "ExternalInput":
            if name != partition_name:
                in_names.append(name)
        elif alloc.kind == "ExternalOutput":
            out_names.append(name)
            out_avals.append(jax.core.ShapedArray(
                tuple(alloc.tensor_shape), mybir.dt.np(alloc.dtype)))
    n_params, n_outs = len(in_names), len(out_avals)
    all_in_names = list(in_names) + list(out_names)
    if partition_name is not None:
        all_in_names.append(partition_name)
    donate = tuple(range(n_params, n_params + n_outs))

    def _body(*args):
        operands = list(args)
        if partition_name is not None:
            operands.append(partition_id_tensor())
        outs = _bass_exec_p.bind(
            *operands, out_avals=tuple(out_avals), in_names=tuple(all_in_names),
            out_names=tuple(out_names), lowering_input_output_aliases=(),
            sim_require_finite=True, sim_require_nnan=True, nc=nc)
        return tuple(outs)

    devices = jax.devices()[:N_CORES]
    mesh = Mesh(np.asarray(devices), ("core",))
    in_specs = (PartitionSpec("core"),) * (n_params + n_outs)
    out_specs = (PartitionSpec("core"),) * n_outs
    fn = jax.jit(
        shard_map(_body, mesh=mesh, in_specs=in_specs, out_specs=out_specs,
                  check_rep=False),
        donate_argnums=donate, keep_unused=True)
    runner = {"fn": fn, "in_names": in_names, "out_names": out_names,
              "out_avals": out_avals, "mesh": mesh}
    _CACHE["runner"] = runner
    return runner


def make_in_maps(x, q_w1, q_b1, q_w2, q_b2, k_w1, k_b1, k_w2, k_b2,
                 v_w1, v_b1, v_w2, v_b2, proj_w, proj_b):
    f32 = np.float32
    x = np.asarray(x, f32)
    shared = {
        "q_w1": np.ascontiguousarray(q_w1, f32),
        "k_w1": np.ascontiguousarray(k_w1, f32),
        "v_w1": np.ascontiguousarray(v_w1, f32),
        "q_w2": np.ascontiguousarray(q_w2, f32),
        "k_w2": np.ascontiguousarray(k_w2, f32),
        "v_w2": np.ascontiguousarray(v_w2, f32),
        "q_b1r": np.ascontiguousarray(np.asarray(q_b1, f32).reshape(HT, P).T),
        "k_b1r": np.ascontiguousarray(np.asarray(k_b1, f32).reshape(HT, P).T),
        "v_b1r": np.ascontiguousarray(np.asarray(v_b1, f32).reshape(HT, P).T),
        "q_b2r": np.ascontiguousarray(np.asarray(q_b2, f32).reshape(CT, P).T),
        "k_b2r": np.ascontiguousarray(np.asarray(k_b2, f32).reshape(CT, P).T),
        "v_b2r": np.ascontiguousarray(np.asarray(v_b2, f32).reshape(1, DIM)),
        "proj_w": np.ascontiguousarray(proj_w, f32),
        "proj_br": np.ascontiguousarray(np.asarray(proj_b, f32).reshape(1, DIM)),
        "ones_r": np.ones((1, P), f32),
    }
    in_maps = []
    for c in range(N_CORES):
        b, half = c // 2, c % 2
        xT_c = np.ascontiguousarray(x[b, half * NT:(half + 1) * NT, :].T)
        in_maps.append({"xT": xT_c, **shared})
    return in_maps


def run_in_maps(in_maps):
    import jax
    from jax.sharding import NamedSharding, PartitionSpec
    r = _get_runner()
    shard = NamedSharding(r["mesh"], PartitionSpec("core"))
    concat_in = [
        np.concatenate([np.asarray(in_maps[c][name]) for c in range(N_CORES)],
                       axis=0)
        for name in r["in_names"]
    ]
    dev_in = [jax.device_put(a, shard) for a in concat_in]
    concat_zeros = [
        np.zeros((N_CORES * av.shape[0], *av.shape[1:]), av.dtype)
        for av in r["out_avals"]
    ]
    out_arrs = r["fn"](*dev_in, *concat_zeros)
    out_arrs = [np.asarray(o) for o in out_arrs]
    return [
        {name: out_arrs[i].reshape(N_CORES, *r["out_avals"][i].shape)[c]
         for i, name in enumerate(r["out_names"])}
        for c in range(N_CORES)
    ]


def kernel(**inputs):
    in_maps = make_in_maps(**inputs)
    results = run_in_maps(in_maps)
    out = np.empty((B, N, DIM), np.float32)
    for c in range(N_CORES):
        b, half = c // 2, c % 2
        out[b, half * NT:(half + 1) * NT, :] = results[c]["out"]
    return out


# revision 13
# speedup vs baseline: 8927.1192x; 8927.1192x over previous
"""FFN-in-head attention Trainium2 kernel (8 NeuronCores, SPMD).

Sharding: core = (batch b, token-half). Each core runs the q/k/v MLPs for its
1024 tokens (channel-major outputs via fp32r matmuls), pairwise-AllGathers
K (channel-major fp32r) and V (token-major bf16) between the two cores of a
batch, computes all 16 heads of attention for its 1024 query tokens
(logits transposed [keys, queries] so the exp'd probabilities are directly in
matmul-stationary layout; softmax denominator via a ones-column appended to V),
and finishes with the output projection (bias folded in as a K=1 matmul).
"""
import sys
sys.path.insert(0, "/opt/trn_rl_repo")
import contextlib
import numpy as np

DIM = 1024
HEADS = 16
HD = 64
HIDDEN = 4096
B = 4
N = 2048
NT = 1024          # tokens per core
P = 128
N_CORES = 8
CH = 2             # token chunks per core in the MLP phase
CHT = NT // CH     # 512 tokens per chunk
HT = HIDDEN // P   # 32 hidden tiles
CT = DIM // P      # 8 channel tiles
KT = N // P        # 16 key tiles
QT = NT // P       # 8 query-token tiles

RG = [[0, 1], [2, 3], [4, 5], [6, 7]]


def build_program(n_cores=N_CORES, with_collectives=True, loop_n=None,
                  sections=("mlp", "attn", "proj")):
    import concourse.bass as bass
    import concourse.mybir as mybir
    import concourse.tile as tile
    from concourse import bacc
    from concourse.masks import make_identity

    fp32 = mybir.dt.float32
    fp32r = mybir.dt.float32r
    bf16 = mybir.dt.bfloat16
    AF = mybir.ActivationFunctionType

    nc = bacc.Bacc("TRN2", target_bir_lowering=False, debug=False,
                   num_devices=n_cores)

    # ---- DRAM I/O ----
    xT = nc.dram_tensor("xT", [DIM, NT], fp32r, kind="ExternalInput").ap()
    w1s, b1s, w2s = {}, {}, {}
    for m in ("q", "k", "v"):
        w1s[m] = nc.dram_tensor(f"{m}_w1", [DIM, HIDDEN], fp32r, kind="ExternalInput").ap()
        b1s[m] = nc.dram_tensor(f"{m}_b1r", [P, HT], fp32, kind="ExternalInput").ap()
        w2s[m] = nc.dram_tensor(f"{m}_w2", [HIDDEN, DIM], fp32r, kind="ExternalInput").ap()
    qb2 = nc.dram_tensor("q_b2r", [P, CT], fp32, kind="ExternalInput").ap()
    kb2 = nc.dram_tensor("k_b2r", [P, CT], fp32, kind="ExternalInput").ap()
    vb2 = nc.dram_tensor("v_b2r", [1, DIM], fp32r, kind="ExternalInput").ap()
    pw = nc.dram_tensor("proj_w", [DIM, DIM], fp32r, kind="ExternalInput").ap()
    pbr = nc.dram_tensor("proj_br", [1, DIM], fp32r, kind="ExternalInput").ap()
    ones_r = nc.dram_tensor("ones_r", [1, P], fp32r, kind="ExternalInput").ap()
    out = nc.dram_tensor("out", [NT, DIM], fp32, kind="ExternalOutput").ap()
    if not with_collectives:
        kfull_in = nc.dram_tensor("kfull", [2 * DIM, NT], fp32r,
                                  kind="ExternalInput").ap()
        vfull_in = nc.dram_tensor("vfull", [2 * NT, DIM], bf16,
                                  kind="ExternalInput").ap()

    def mlp_phase(tc, psp, outer_t, dram_t):
        (xT_sb, qT_sb, ones_sb, vb2_sb, pbr_sb) = outer_t
        (ag_k_in, ag_v_in, ag_k_out, ag_v_out) = dram_t
        with tc.tile_pool(name="mlp", bufs=1) as mlp, \
             tc.tile_pool(name="w1p", bufs=2) as w1p, \
             tc.tile_pool(name="w2p", bufs=2) as w2p, \
             tc.tile_pool(name="w2vp", bufs=3) as w2vp, \
             tc.tile_pool(name="stg", bufs=3) as stg:
            for m in ("k", "v", "q"):
                b1_sb = mlp.tile([P, HT], fp32, tag="b1", name="b1")
                nc.sync.dma_start(b1_sb[:], b1s[m][:])
                if m in ("k", "q"):
                    b2_sb = mlp.tile([P, CT], fp32, tag="b2", name="b2")
                    nc.sync.dma_start(b2_sb[:], (kb2 if m == "k" else qb2)[:])
                w1_r = w1s[m].rearrange("(kc p) h -> p kc h", p=P)
                w2_r = w2s[m].rearrange("(kh p) c -> p kh c", p=P)

                for ch in range(CH):
                    tok = slice(ch * CHT, (ch + 1) * CHT)
                    # fc1: hT[h, tok_chunk] = gelu(w1.T @ x + b1)
                    hT_sb = mlp.tile([P, HT, CHT], fp32r, tag="hT", name="hT")
                    for ht in range(HT):
                        w1t = w1p.tile([P, CT, P], fp32r, tag="w1t", name="w1t")
                        nc.sync.dma_start(
                            w1t[:], w1_r[:, :, ht * P:(ht + 1) * P])
                        ps = psp.tile([P, CHT], fp32, tag="ps", name="ps1")
                        for kc in range(CT):
                            nc.tensor.matmul(
                                ps[:], w1t[:, kc, :], xT_sb[:, kc, tok],
                                start=(kc == 0), stop=(kc == CT - 1))
                        nc.scalar.activation(
                            hT_sb[:, ht, :], ps[:], AF.Gelu,
                            bias=b1_sb[:, ht:ht + 1])

                    if m in ("k", "q"):
                        # fc2 channel-major: y[c, tok] = w2.T @ h + b2
                        for ct in range(CT):
                            ps = psp.tile([P, CHT], fp32, tag="ps", name="ps2")
                            for khh in range(2):
                                w2t = w2p.tile([P, HT // 2, P], fp32r,
                                               tag="w2t", name="w2t")
                                nc.sync.dma_start(
                                    w2t[:],
                                    w2_r[:, khh * (HT // 2):(khh + 1) * (HT // 2),
                                         ct * P:(ct + 1) * P])
                                for kh in range(HT // 2):
                                    k_abs = khh * (HT // 2) + kh
                                    nc.tensor.matmul(
                                        ps[:], w2t[:, kh, :], hT_sb[:, k_abs, :],
                                        start=(k_abs == 0), stop=(k_abs == HT - 1))
                            if m == "q":
                                nc.scalar.activation(
                                    qT_sb[:, ct, tok], ps[:], AF.Identity,
                                    bias=b2_sb[:, ct:ct + 1])
                            else:
                                kst = stg.tile([P, CHT], fp32r, tag="kst", name="kst")
                                nc.scalar.activation(
                                    kst[:], ps[:], AF.Identity,
                                    bias=b2_sb[:, ct:ct + 1])
                                nc.sync.dma_start(
                                    ag_k_in[ct * P:(ct + 1) * P, tok], kst[:])
                    else:
                        # fc2 token-major (v): y[tok, c] = h.T @ w2 + b2
                        TT = CHT // P
                        for nt in range(2):
                            cs = slice(nt * 512, (nt + 1) * 512)
                            pss = [psp.tile([P, 512], fp32, tag="ps",
                                            name=f"psv{tt}")
                                   for tt in range(TT)]
                            for kh in range(HT):
                                w2vt = w2vp.tile([P, 512], fp32r, tag="w2vt",
                                                 name="w2vt")
                                nc.sync.dma_start(w2vt[:], w2_r[:, kh, cs])
                                for tt in range(TT):
                                    nc.tensor.matmul(
                                        pss[tt][:],
                                        hT_sb[:, kh, tt * P:(tt + 1) * P],
                                        w2vt[:], start=(kh == 0), stop=False,
                                        skip_group_check=True)
                            for tt in range(TT):
                                nc.tensor.matmul(
                                    pss[tt][:], ones_sb[:, :], vb2_sb[:, cs],
                                    start=False, stop=True,
                                    skip_group_check=True)
                                vst = stg.tile([P, 512], bf16, tag="vst",
                                               name="vst")
                                nc.vector.tensor_copy(vst[:], pss[tt][:])
                                nc.sync.dma_start(
                                    ag_v_in[ch * CHT + tt * P:
                                            ch * CHT + (tt + 1) * P, cs],
                                    vst[:])

                if with_collectives and m == "k":
                    nc.gpsimd.collective_compute(
                        "AllGather", mybir.AluOpType.bypass, replica_groups=RG,
                        ins=[ag_k_in.opt()], outs=[ag_k_out.opt()])
                elif with_collectives and m == "v":
                    nc.gpsimd.collective_compute(
                        "AllGather", mybir.AluOpType.bypass, replica_groups=RG,
                        ins=[ag_v_in.opt()], outs=[ag_v_out.opt()])

    def attn_phase(tc, psp, psp2, outer_t, dram_t, aoT_sb):
        # attn@v uses V as the (padded, M=128) stationary operand:
        #   head A: [v_A | 1 | 0...] -> psum rows 0-63 = out, row 64 = denom
        #   head B: [0... | 1 | v_B] -> psum row 63 = denom, rows 64-127 = out
        # so the per-head outputs land channel-major at the right partitions
        # for the projection, with the softmax denominators in spare rows.
        (xT_sb, qT_sb, ones_sb, vb2_sb, pbr_sb) = outer_t
        (ag_k_in, ag_v_in, ag_k_out, ag_v_out) = dram_t
        k_r = ag_k_out.rearrange("(half ct p) t -> p ct half t",
                                 half=2, ct=CT, p=P)
        v_r = ag_v_out.rearrange("(ktl p) c -> p ktl c", p=P)
        with tc.tile_pool(name="attn", bufs=2) as atp, \
             tc.tile_pool(name="attbig", bufs=1) as atbig, \
             tc.tile_pool(name="attdn", bufs=1) as dnp, \
             tc.tile_pool(name="attnd", bufs=2, space="DRAM") as dnd:
            for hp in range(HEADS // 2):
                k2 = atp.tile([P, 2, NT], fp32r, tag="k2", name="k2")
                nc.sync.dma_start(k2[:], k_r[:, hp])
                vA = atp.tile([P, KT, P], bf16, tag="vA", name="vA")
                vB = atp.tile([P, KT, P], bf16, tag="vB", name="vB")
                cA, cB = 2 * hp * HD, (2 * hp + 1) * HD
                nc.gpsimd.memset(vA[:, :, HD:], 0.0)
                nc.gpsimd.memset(vA[:, :, HD:HD + 1], 1.0)
                nc.sync.dma_start(vA[:, :, 0:HD], v_r[:, :, cA:cA + HD])
                nc.gpsimd.memset(vB[:, :, 0:HD], 0.0)
                nc.gpsimd.memset(vB[:, :, 0:1], 1.0)
                nc.sync.dma_start(vB[:, :, HD:], v_r[:, :, cB:cB + HD])

                atA = atbig.tile([P, KT, NT], bf16, tag="atA", name="atA")
                atB = atbig.tile([P, KT, NT], bf16, tag="atB", name="atB")
                for ktl in range(KT):
                    half, col = ktl // CT, (ktl % CT) * P
                    psA = psp2.tile([P, NT], fp32, tag="ps2", name="psA")
                    psB = psp2.tile([P, NT], fp32, tag="ps2", name="psB")
                    for nt in range(2):
                        qs = slice(nt * 512, (nt + 1) * 512)
                        nc.tensor.matmul(
                            psA[:, qs], k2[0:HD, half, col:col + P],
                            qT_sb[0:HD, hp, qs], start=True, stop=True,
                            skip_group_check=True)
                        nc.tensor.matmul(
                            psB[:, qs], k2[HD:P, half, col:col + P],
                            qT_sb[HD:P, hp, qs], start=True, stop=True,
                            skip_group_check=True)
                    nc.scalar.activation(atA[:, ktl, :], psA[:], AF.Exp,
                                         scale=float(HD) ** -0.5)
                    nc.scalar.activation(atB[:, ktl, :], psB[:], AF.Exp,
                                         scale=float(HD) ** -0.5)

                # attn@v: stationary = padded v tiles, moving = probabilities.
                # Head A denom lands at psum partition 64, head B at 63.
                pavs = {}
                for head, at, vv in ((0, atA, vA), (1, atB, vB)):
                    for nt in range(2):
                        qs = slice(nt * 512, (nt + 1) * 512)
                        pav = psp.tile([P, 512], fp32, tag="ps", name="pav")
                        for ktl in range(KT):
                            nc.tensor.matmul(
                                pav[:], vv[:, ktl, :], at[:, ktl, qs],
                                start=(ktl == 0), stop=(ktl == KT - 1))
                        pavs[(head, nt)] = pav

                # denominators: head A at partition HD, head B at partition 0.
                # reciprocal (single DVE op over rows 0..HD), bounce the two
                # rows through DRAM, and DMA-broadcast to 64 partitions each.
                dn = dnp.tile([P, 2, 512], fp32, tag="dn", name="dn")
                nc.gpsimd.memset(dn[:, :, :], 1.0)
                for nt in range(2):
                    nc.vector.tensor_copy(dn[HD:HD + 1, nt, :],
                                          pavs[(0, nt)][HD:HD + 1, :])
                    nc.vector.tensor_copy(dn[0:1, nt, :],
                                          pavs[(1, nt)][0:1, :])
                rec = dnp.tile([P, 2, 512], fp32, tag="rec", name="rec")
                nc.vector.reciprocal(rec[0:HD + 1, :, :], dn[0:HD + 1, :, :])
                drec = dnd.tile([2, 2, 512], fp32, tag="drec", name="drec")
                nc.sync.dma_start(drec[0:1, :, :], rec[HD:HD + 1, :, :])
                nc.sync.dma_start(drec[1:2, :, :], rec[0:1, :, :])
                rd = dnp.tile([P, 2, 512], fp32, tag="rd", name="rd")
                nc.sync.dma_start(rd[0:HD, :, :],
                                  drec[0:1, :, :].broadcast_to([HD, 2, 512]))
                nc.sync.dma_start(rd[HD:P, :, :],
                                  drec[1:2, :, :].broadcast_to([HD, 2, 512]))
                for (head, nt), pav in pavs.items():
                    qs = slice(nt * 512, (nt + 1) * 512)
                    rows = slice(0, HD) if head == 0 else slice(HD, P)
                    nc.vector.tensor_mul(
                        aoT_sb[rows, hp, qs], pav[rows, :], rd[rows, nt, :])

    def proj_phase(tc, psp, outer_t, aoT_sb):
        (xT_sb, qT_sb, ones_sb, vb2_sb, pbr_sb) = outer_t
        with tc.tile_pool(name="proj", bufs=16) as pjp:
            pwt = {}
            for nt in range(2):
                for kc in range(CT):
                    t = pjp.tile([P, 512], fp32r, tag="pwt", name="pwt")
                    nc.sync.dma_start(
                        t[:], pw[kc * P:(kc + 1) * P, nt * 512:(nt + 1) * 512])
                    pwt[(nt, kc)] = t
            for tt in range(QT):
                for nt in range(2):
                    cs = slice(nt * 512, (nt + 1) * 512)
                    ps = psp.tile([P, 512], fp32, tag="ps", name="psp")
                    for kc in range(CT):
                        nc.tensor.matmul(
                            ps[:], aoT_sb[:, kc, tt * P:(tt + 1) * P],
                            pwt[(nt, kc)][:], start=(kc == 0), stop=False,
                            skip_group_check=True)
                    nc.tensor.matmul(
                        ps[:], ones_sb[:, :], pbr_sb[:, cs],
                        start=False, stop=True, skip_group_check=True)
                    ot = pjp.tile([P, 512], fp32, tag="ot", bufs=3, name="ot")
                    nc.vector.tensor_copy(ot[:], ps[:])
                    nc.sync.dma_start(out[tt * P:(tt + 1) * P, cs], ot[:])

    with tile.TileContext(nc) as tc:
        loop_ctx = tc.For_i(0, loop_n, 1) if loop_n else contextlib.nullcontext()
        with loop_ctx, \
             tc.tile_pool(name="outer", bufs=1) as outer, \
             tc.tile_pool(name="ps", bufs=4, space="PSUM") as psp, \
             tc.tile_pool(name="ps2", bufs=2, space="PSUM") as psp2, \
             tc.tile_pool(name="dram", bufs=1, space="DRAM") as dram:

            xT_sb = outer.tile([P, CT, NT], fp32r, tag="big", name="xTs")
            nc.sync.dma_start(xT_sb[:], xT.rearrange("(ct p) t -> p ct t", p=P))
            qT_sb = outer.tile([P, CT, NT], fp32r, tag="qT", name="qTs")
            ones_sb = outer.tile([1, P], fp32r, tag="ones", name="oness")
            nc.sync.dma_start(ones_sb[:], ones_r[:])
            vb2_sb = outer.tile([1, DIM], fp32r, tag="vb2", name="vb2s")
            nc.sync.dma_start(vb2_sb[:], vb2[:])
            pbr_sb = outer.tile([1, DIM], fp32r, tag="pbr", name="pbrs")
            nc.sync.dma_start(pbr_sb[:], pbr[:])
            outer_t = (xT_sb, qT_sb, ones_sb, vb2_sb, pbr_sb)

            ag_k_in = dram.tile([DIM, NT], fp32r, tag="agki", name="agki")
            ag_v_in = dram.tile([NT, DIM], bf16, tag="agvi", name="agvi")
            if with_collectives:
                ag_k_out = dram.tile([2 * DIM, NT], fp32r, tag="agko", name="agko")
                ag_v_out = dram.tile([2 * NT, DIM], bf16, tag="agvo", name="agvo")
            else:
                ag_k_out, ag_v_out = kfull_in, vfull_in
            dram_t = (ag_k_in, ag_v_in, ag_k_out, ag_v_out)

            if "mlp" in sections:
                mlp_phase(tc, psp, outer_t, dram_t)
            aoT_sb = outer.tile([P, CT, NT], fp32r, tag="big", name="aoTs")
            if "attn" in sections:
                attn_phase(tc, psp, psp2, outer_t, dram_t, aoT_sb)
            if "proj" in sections:
                proj_phase(tc, psp, outer_t, aoT_sb)
            else:
                # dummy: route something to `out` so it is written
                ot0 = outer.tile([P, 512], fp32, tag="ot0", name="ot0")
                src = aoT_sb if "attn" in sections else qT_sb
                nc.vector.tensor_copy(ot0[:], src[:, 0, 0:512])
                nc.sync.dma_start(out[0:P, 0:512], ot0[:])

    nc.compile()
    return nc


_CACHE = {}


def _get_runner():
    if "runner" in _CACHE:
        return _CACHE["runner"]
    import jax
    from jax.sharding import Mesh, PartitionSpec
    from jax.experimental.shard_map import shard_map
    from concourse import mybir
    from concourse.bass2jax import (_bass_exec_p, partition_id_tensor,
                                    install_neuronx_cc_hook)

    nc = build_program()
    install_neuronx_cc_hook()
    partition_name = nc.partition_id_tensor.name if nc.partition_id_tensor else None
    in_names, out_names, out_avals = [], [], []
    for alloc in nc.m.functions[0].allocations:
        if not isinstance(alloc, mybir.MemoryLocationSet):
            continue
        name = alloc.memorylocations[0].name
        if alloc.kind == "ExternalInput":
            if name != partition_name:
                in_names.append(name)
        elif alloc.kind == "ExternalOutput":
            out_names.append(name)
            out_avals.append(jax.core.ShapedArray(
                tuple(alloc.tensor_shape), mybir.dt.np(alloc.dtype)))
    n_params, n_outs = len(in_names), len(out_avals)
    all_in_names = list(in_names) + list(out_names)
    if partition_name is not None:
        all_in_names.append(partition_name)
    donate = tuple(range(n_params, n_params + n_outs))

    def _body(*args):
        operands = list(args)
        if partition_name is not None:
            operands.append(partition_id_tensor())
        outs = _bass_exec_p.bind(
            *operands, out_avals=tuple(out_avals), in_names=tuple(all_in_names),
            out_names=tuple(out_names), lowering_input_output_aliases=(),
            sim_require_finite=True, sim_require_nnan=True, nc=nc)
        return tuple(outs)

    devices = jax.devices()[:N_CORES]
    mesh = Mesh(np.asarray(devices), ("core",))
    in_specs = (PartitionSpec("core"),) * (n_params + n_outs)
    out_specs = (PartitionSpec("core"),) * n_outs
    fn = jax.jit(
        shard_map(_body, mesh=mesh, in_specs=in_specs, out_specs=out_specs,
                  check_rep=False),
        donate_argnums=donate, keep_unused=True)
    runner = {"fn": fn, "in_names": in_names, "out_names": out_names,
              "out_avals": out_avals, "mesh": mesh}
    _CACHE["runner"] = runner
    return runner


def make_in_maps(x, q_w1, q_b1, q_w2, q_b2, k_w1, k_b1, k_w2, k_b2,
                 v_w1, v_b1, v_w2, v_b2, proj_w, proj_b):
    f32 = np.float32
    x = np.asarray(x, f32)
    shared = {
        "q_w1": np.ascontiguousarray(q_w1, f32),
        "k_w1": np.ascontiguousarray(k_w1, f32),
        "v_w1": np.ascontiguousarray(v_w1, f32),
        "q_w2": np.ascontiguousarray(q_w2, f32),
        "k_w2": np.ascontiguousarray(k_w2, f32),
        "v_w2": np.ascontiguousarray(v_w2, f32),
        "q_b1r": np.ascontiguousarray(np.asarray(q_b1, f32).reshape(HT, P).T),
        "k_b1r": np.ascontiguousarray(np.asarray(k_b1, f32).reshape(HT, P).T),
        "v_b1r": np.ascontiguousarray(np.asarray(v_b1, f32).reshape(HT, P).T),
        "q_b2r": np.ascontiguousarray(np.asarray(q_b2, f32).reshape(CT, P).T),
        "k_b2r": np.ascontiguousarray(np.asarray(k_b2, f32).reshape(CT, P).T),
        "v_b2r": np.ascontiguousarray(np.asarray(v_b2, f32).reshape(1, DIM)),
        "proj_w": np.ascontiguousarray(proj_w, f32),
        "proj_br": np.ascontiguousarray(np.asarray(proj_b, f32).reshape(1, DIM)),
        "ones_r": np.ones((1, P), f32),
    }
    in_maps = []
    for c in range(N_CORES):
        b, half = c // 2, c % 2
        xT_c = np.ascontiguousarray(x[b, half * NT:(half + 1) * NT, :].T)
        in_maps.append({"xT": xT_c, **shared})
    return in_maps


def run_in_maps(in_maps):
    import jax
    from jax.sharding import NamedSharding, PartitionSpec
    r = _get_runner()
    shard = NamedSharding(r["mesh"], PartitionSpec("core"))
    concat_in = [
        np.concatenate([np.asarray(in_maps[c][name]) for c in range(N_CORES)],
                       axis=0)
        for name in r["in_names"]
    ]
    dev_in = [jax.device_put(a, shard) for a in concat_in]
    concat_zeros = [
        np.zeros((N_CORES * av.shape[0], *av.shape[1:]), av.dtype)
        for av in r["out_avals"]
    ]
    out_arrs = r["fn"](*dev_in, *concat_zeros)
    out_arrs = [np.asarray(o) for o in out_arrs]
    return [
        {name: out_arrs[i].reshape(N_CORES, *r["out_avals"][i].shape)[c]
         for i, name in enumerate(r["out_names"])}
        for c in range(N_CORES)
    ]


def kernel(**inputs):
    in_maps = make_in_maps(**inputs)
    results = run_in_maps(in_maps)
    out = np.empty((B, N, DIM), np.float32)
    for c in range(N_CORES):
        b, half = c // 2, c % 2
        out[b, half * NT:(half + 1) * NT, :] = results[c]["out"]
    return out
